# revision 51
# baseline (speedup 1.0000x reference)
"""Causal multi-head attention on 8 TRN2 NeuronCores.

Reference computation (fp32):
    q,k,v = x @ {Q,K,V}.T split into 16 heads of 64
    scores = q k^T / 8, causal mask, softmax
    out    = (attn @ v concat heads) @ W_o.T

Sharding: core c (0..7) takes batch b = c//4 and head group g = c%4
(heads 4g..4g+3, i.e. a 256-row slice of Q/K/V and a 256-column slice
of W_o). Each core produces a partial [T, D] output; the host sums the
4 partials per batch. No on-device collectives.

Per-core DRAM layout (host pre-packs everything so every matmul
contraction dim lands on SBUF partitions AND every SBUF tile loads with
ONE wide dma_start -- DMA trigger instructions cost ~650ns each on the
issuing engine queue, so few big triggers beat many small ones):
    xp     [128, 16384] x[b].T packed chunk-major then db-major:
                        xp[p, 4096c + 512db + t] = x[b].T[128db+p, 512c+t]
    wqp    [128, 2048]  wqp[p, 256db+e] = Q[slice].T[128db+p, e]
    wkp    [128, 2048]  same for K
    wvp    [128, 2048]  same for V
    wop    [128, 2048]  wop[p, 1024db+e] = W_o[:, slice].T[128db+p, e]
    maskz  [128, 128]   triangular f >= p mask ([tk, tq] orientation)
    ones64 [128, 64]    all-ones (denominator matmul weights)

Input DMA triggers are split across the two HWDGE queues (Sync + ACT)
so the queues stream in parallel from t~8us; the non-critical bulk
(x chunks 1-3, W_o) is held back behind a write-after-read gate so it
cannot steal HBM bandwidth from the stage-1-chunk-0 critical stream.
Output partials are written bf16 (host sums in fp32), halving the
8MB/core output traffic.

Attention is computed transposed (ST[tk, tq] = k-block . qT-chunk) so
softmax exp is elementwise (no max subtraction: scores ~ N(0,1), exp
cannot overflow) and PV needs no transposes; exp runs on ACT straight
out of PSUM.

Heads are processed in PAIRS (2p, 2p+1) to exploit PE array tiling:
  - ST: the two heads' K slices live at partitions 0-63 / 64-127 of the
    same kT tile, so the two 64-contraction matmuls land in different
    PE row groups and run CONCURRENTLY (row tiling).
  - PV: each head's output is 64 rows, written to acc[0:64]/acc[64:128]
    -> different PE col groups, also concurrent (col tiling).
  - denominator: matmul with an all-ones [128, 64] lhsT per head: every
    output row gets sum_tk(e), i.e. the softmax denominator REPLICATED
    across 64 partitions -- den pair is col-tiled/concurrent too, and
    normalization needs no partition broadcast at all: one elementwise
    reciprocal of the [128, 512] den tile + one multiply per pair.
  - exp: one ACT instruction per (pair, j) covering both heads' score
    tiles (the pair's two [128,512] psum tiles are one [128,1024]
    tile); full-width even on diagonal blocks (the below-diagonal cols
    hold bounded stale scores, and PV only reads cols >= off).

The schedule keeps the PE stream dense so the HAM clock gate stays at
K=8/8 (idle gaps > ~3us re-throttle the PE to half clock): PV lags ST
by two j-steps (hiding exp latency), stage-1 / stage-5 matmul groups
are interleaved INTO each pair's j-loop as filler units so the PE
never waits for the ACT exp stream, matmuls are issued in same-tiling-
mode bursts (the PE drains ~160ns at each row/col-tiling mode switch),
and dummy matmul bursts bridge the DMA-wait gaps of the startup phase
so HAM is warm by ~17us instead of ~24us.
"""

import numpy as np

import concourse.bass as bass  # noqa: F401
import concourse.tile as tile
from concourse import bacc, mybir
from concourse.bass_utils import run_bass_kernel_spmd

F32 = mybir.dt.float32
F32R = mybir.dt.float32r
BF16 = mybir.dt.bfloat16
EXP = mybir.ActivationFunctionType.Exp

import os as _os

WDT = BF16 if _os.environ.get("MHA_DTYPE", "bf16") == "bf16" else F32R
# output partials in bf16 halves the 8MB/core output DMA; the host sums
# the 4 partials per batch in fp32. Costs ~2x on the (still ~5x under
# budget) relative error.
ODT = BF16 if _os.environ.get("MHA_OUT", "bf16") == "bf16" else F32
ONP = np.float32 if ODT is F32 else None

N_CORES = 8
T = 2048          # sequence length
D = 1024          # model dim
HPC = 4           # heads per core
HD = 64           # head dim
DS = HPC * HD     # 256: per-core slice of D
CH = 512          # tq chunk width
NCH = T // CH     # chunks
NTB = T // 128    # 128-row t blocks
NDB = D // 128    # 128-row d blocks


def build_program():
    nc = bacc.Bacc("TRN2", target_bir_lowering=False, debug=False,
                   num_devices=N_CORES)
    xp_d = nc.dram_tensor("xp", [128, 16384], WDT, kind="ExternalInput").ap()
    wqp_d = nc.dram_tensor("wqp", [128, 2048], WDT, kind="ExternalInput").ap()
    wkp_d = nc.dram_tensor("wkp", [128, 2048], WDT, kind="ExternalInput").ap()
    wvp_d = nc.dram_tensor("wvp", [128, 2048], WDT, kind="ExternalInput").ap()
    wop_d = nc.dram_tensor("wop", [128, 2048], WDT, kind="ExternalInput").ap()
    maskz_d = nc.dram_tensor("maskz", [128, 128], WDT,
                             kind="ExternalInput").ap()
    out_d = nc.dram_tensor("out", [T, D], ODT, kind="ExternalOutput").ap()

    with tile.TileContext(nc) as tc, \
         tc.tile_pool(name="xt", bufs=1) as xt_pool, \
         tc.tile_pool(name="wq", bufs=1) as wq_pool, \
         tc.tile_pool(name="wk", bufs=1) as wk_pool, \
         tc.tile_pool(name="wv", bufs=1) as wv_pool, \
         tc.tile_pool(name="wo", bufs=1) as wo_pool, \
         tc.tile_pool(name="cst", bufs=1) as cst_pool, \
         tc.tile_pool(name="qk", bufs=16) as qk_pool, \
         tc.tile_pool(name="vv", bufs=16) as vv_pool, \
         tc.tile_pool(name="ot", bufs=8) as ot_pool, \
         tc.tile_pool(name="ee", bufs=6) as e_pool, \
         tc.tile_pool(name="rb", bufs=2) as rb_pool, \
         tc.tile_pool(name="ob", bufs=4) as ob_pool:

        # ---- input DMAs: wide triggers split across both HWDGE queues.
        # sync carries what stage-1 chunk 0 consumes, in consumption
        # order with small first bites; scalar (idle until the exp
        # stream starts) carries the rest, gated behind a tiny copy that
        # waits for sync's last critical load so the bulk doesn't steal
        # HBM bandwidth from the startup-critical stream.
        # ones64 is synthesized on-chip (gpsimd memset right after the
        # preamble) so the HAM warmup spin can start ~2us before the
        # first DMA lands.
        ones64_t = cst_pool.tile([128, 64], WDT, tag="ones64")
        nc.gpsimd.memset(ones64_t[:], 1.0)
        wq_t = wq_pool.tile([128, 2048], WDT, tag="wq")
        xt_t = xt_pool.tile([128, 16384], WDT, tag="xt")
        wk_t = wk_pool.tile([128, 2048], WDT, tag="wk")
        wv_t = wv_pool.tile([128, 2048], WDT, tag="wv")
        nc.sync.dma_start(wq_t[:, 0:256], wqp_d[:, 0:256])
        nc.sync.dma_start(xt_t[:, 0:512], xp_d[:, 0:512])
        nc.sync.dma_start(wq_t[:, 256:512], wqp_d[:, 256:512])
        nc.sync.dma_start(xt_t[:, 512:1024], xp_d[:, 512:1024])
        nc.sync.dma_start(wk_t[:, 0:256], wkp_d[:, 0:256])
        nc.sync.dma_start(wq_t[:, 512:2048], wqp_d[:, 512:2048])
        nc.sync.dma_start(xt_t[:, 1024:2048], xp_d[:, 1024:2048])
        nc.sync.dma_start(wk_t[:, 256:2048], wkp_d[:, 256:2048])
        nc.sync.dma_start(xt_t[:, 2048:3072], xp_d[:, 2048:3072])
        nc.sync.dma_start(xt_t[:, 3072:4096], xp_d[:, 3072:4096])
        nc.sync.dma_start(wv_t[:], wvp_d[:])

        maskz_t = cst_pool.tile([128, 128], WDT, tag="maskz")
        nc.scalar.dma_start(maskz_t[:], maskz_d[:])
        # gate the bulk loads behind the startup-critical stream: a tiny
        # read of each bulk destination region, data-dependent on the
        # tail of wk, forces a write-after-read delay on the bulk DMAs
        # (the scheduler's priority heap would otherwise start them
        # immediately and steal HBM bandwidth from stage-1 chunk 0).
        wo_t = wo_pool.tile([128, 2048], WDT, tag="wo")
        for i, probe in enumerate((xt_t[0:1, 4096:4100],
                                   xt_t[0:1, 8192:8196],
                                   xt_t[0:1, 12288:12292],
                                   wo_t[0:1, 0:4])):
            g = cst_pool.tile([1, 4], WDT, tag=f"gate{i}")
            nc.vector.tensor_add(g[:], probe, wk_t[0:1, 2044:2048])
        nc.scalar.dma_start(xt_t[:, 4096:8192], xp_d[:, 4096:8192])
        nc.scalar.dma_start(wo_t[:], wop_d[:])
        nc.scalar.dma_start(xt_t[:, 8192:12288], xp_d[:, 8192:12288])
        nc.scalar.dma_start(xt_t[:, 12288:16384], xp_d[:, 12288:16384])

        def xt_s(tch, db):  # x chunk tch, 128-row d block db: [128, 512]
            o = 4096 * tch + 512 * db
            return xt_t[:, o:o + 512]

        def wq_s(db):
            return wq_t[:, 256 * db:256 * db + 256]

        def wk_s(db):
            return wk_t[:, 256 * db:256 * db + 256]

        def wv_s(db):
            return wv_t[:, 256 * db:256 * db + 256]

        def wo_s(db):
            return wo_t[:, 1024 * db:1024 * db + 1024]

        # persistent E tiles: [128, 1024] = both heads of a pair, one j
        e_tiles = [e_pool.tile([128, 2 * CH], WDT, tag="ee", name=f"ee{i}")
                   for i in range(6)]

        qT_t = [[None] * NCH for _ in range(2)]
        kT_t = [[None] * NCH for _ in range(2)]
        v_t = [None] * NTB

        # oT_t[p][c]: [128, CH] normalized attention outputs for head
        # pair p (head 2p rows 0-63, head 2p+1 rows 64-127), chunk c
        oT_t = [[ot_pool.tile([128, CH], WDT, tag="ot", name=f"ot{p}_{c}")
                 for c in range(NCH)] for p in range(2)]
        state = {"eidx": 0}

        with tc.tile_pool(name="pst", bufs=2, space="PSUM") as pst_pool, \
             tc.tile_pool(name="pac", bufs=1, space="PSUM") as pac_pool, \
             tc.tile_pool(name="pdn", bufs=1, space="PSUM") as pdn_pool, \
             tc.tile_pool(name="ps1", bufs=2, space="PSUM") as ps1_pool:

            def emit_qk_group(tch, eb, wsl, dst):
                ps = ps1_pool.tile([128, 512], F32, tag="ps1",
                                   name=f"p1_{tch}_{eb}_{dst is kT_t}")
                for db in range(NDB):
                    nc.tensor.matmul(
                        ps[:], wsl(db)[:, 128 * eb:128 * eb + 128],
                        xt_s(tch, db), start=(db == 0), stop=(db == NDB - 1))
                q = qk_pool.tile([128, 512], WDT, tag="qk",
                                 name=f"qk_{tch}_{eb}_{dst is kT_t}")
                nc.vector.tensor_copy(q[:], ps[:])
                dst[eb][tch] = q

            def emit_v_group(tb):
                ps = ps1_pool.tile([128, 256], F32, tag="ps1",
                                   name=f"p1v_{tb}")
                for db in range(NDB):
                    nc.tensor.matmul(
                        ps[:],
                        xt_s(tb // 4, db)[:, 128 * (tb % 4):128 * (tb % 4) + 128],
                        wv_s(db), start=(db == 0), stop=(db == NDB - 1))
                v = vv_pool.tile([128, 256], WDT, tag="vv", name=f"v{tb}")
                nc.vector.tensor_copy(v[:], ps[:])
                v_t[tb] = v

            def emit_pv(acc, dn, p, jmax, j, e, off, eb):
                # PV pair: col groups 0-1 (head 2p) and 2-3 (head 2p+1)
                # run concurrently; denominator pair likewise. eb is the
                # j's base offset (0 or 2*CH) inside the block e-tile.
                st, sp = (j == 0), (j == jmax)
                nc.tensor.matmul(
                    acc[0:64, off:CH], v_t[j][:, 128 * p:128 * p + 64],
                    e[:, eb + off:eb + CH], start=st, stop=sp,
                    skip_group_check=True)
                nc.tensor.matmul(
                    acc[64:128, off:CH],
                    v_t[j][:, 128 * p + 64:128 * p + 128],
                    e[:, eb + CH + off:eb + 2 * CH], start=st, stop=sp,
                    skip_group_check=True)
                nc.tensor.matmul(
                    dn[0:64, off:CH], ones64_t[:], e[:, eb + off:eb + CH],
                    start=st, stop=sp, skip_group_check=True)
                nc.tensor.matmul(
                    dn[64:128, off:CH], ones64_t[:],
                    e[:, eb + CH + off:eb + 2 * CH],
                    start=st, stop=sp, skip_group_check=True)

            def emit_pair(c, p, fillers):
                # attention for one (chunk, head-pair), in 2-j bursts:
                # [ST pack, ST pack] (row-tiled mode) then the previous
                # burst's [PV+den packs] (col-tiled mode), so the PE
                # pays the tiling-mode-switch drain (~160ns) twice per
                # burst instead of thrice per j. Filler units keep the
                # PE fed while ACT works through the exp backlog.
                jmax = 4 * c + 3
                acc = pac_pool.tile([128, CH], F32, tag="pac",
                                    name=f"ac{c}_{p}")
                dn = pdn_pool.tile([128, CH], F32, tag="pdn",
                                   name=f"dn{c}_{p}")
                nf = len(fillers)
                fired = 0
                prevs = []
                for jb in range(0, jmax + 1, 2):
                    js = [j for j in (jb, jb + 1) if j <= jmax]
                    cur = []
                    for j in js:
                        off = max(0, 128 * j - CH * c)
                        stp = pst_pool.tile([128, 2 * CH], F32, tag="pst",
                                            name=f"st{c}_{p}_{j}")
                        # ST pair: row groups 0-63 / 64-127, concurrent
                        nc.tensor.matmul(
                            stp[:, off:CH],
                            kT_t[p][j // 4][
                                0:64, 128 * (j % 4):128 * (j % 4) + 128],
                            qT_t[p][c][0:64, off:CH],
                            start=True, stop=True)
                        nc.tensor.matmul(
                            stp[:, CH + off:2 * CH],
                            kT_t[p][j // 4][
                                64:128, 128 * (j % 4):128 * (j % 4) + 128],
                            qT_t[p][c][64:128, off:CH],
                            start=True, stop=True)
                        cur.append((j, stp, off))
                    pend = []
                    for j, stp, off in cur:
                        e = e_tiles[state["eidx"] % len(e_tiles)]
                        state["eidx"] += 1
                        # one exp for both heads, full width: ACT
                        # per-instruction overhead (~460ns incl
                        # semaphore wait) dominates any trimming of the
                        # below-diagonal cols (bounded stale scores; PV
                        # reads >= off).
                        nc.scalar.activation(e[:], stp[:], EXP,
                                             scale=0.125)
                        if j >= 4 * c:
                            with tc.high_priority():
                                nc.vector.tensor_mul(
                                    e[:, off:off + 128],
                                    e[:, off:off + 128], maskz_t[:])
                                nc.vector.tensor_mul(
                                    e[:, CH + off:CH + off + 128],
                                    e[:, CH + off:CH + off + 128],
                                    maskz_t[:])
                        pend.append((j, e, off, 0))
                    for blk in prevs:
                        for jd, ed, ad, ebd in blk:
                            emit_pv(acc, dn, p, jmax, jd, ed, ad, ebd)
                    prevs = [pend]
                    want = ((jb + 2) * nf) // (jmax + 2)
                    while fired < want:
                        fillers[fired]()
                        fired += 1
                for blk in prevs:
                    for jd, ed, ad, ebd in blk:
                        emit_pv(acc, dn, p, jmax, jd, ed, ad, ebd)
                while fired < nf:
                    fillers[fired]()
                    fired += 1
                # normalization: the den matmuls left each head's
                # denominator replicated across its 64 partitions, so
                # this is just an elementwise reciprocal + multiply.
                rbt = rb_pool.tile([128, CH], F32, tag="rb")
                with tc.high_priority():
                    nc.vector.reciprocal_approx_fast(rbt[:], dn[:])
                    nc.vector.tensor_mul(oT_t[p][c][:], acc[:], rbt[:])

            def emit_stage5(ps_pool, tb, cp_engines, split_dma=False):
                c, tw = tb // 4, 128 * (tb % 4)
                pss = [ps_pool.tile([128, 512], F32, tag="ps1",
                                    name=f"ps5_{tb}_{eb}")
                       for eb in range(2)]
                for db in range(2):
                    for eb in range(2):
                        nc.tensor.matmul(
                            pss[eb][:], oT_t[db][c][:, tw:tw + 128],
                            wo_s(db)[:, 512 * eb:512 * eb + 512],
                            start=(db == 0), stop=(db == 1))
                ob = ob_pool.tile([128, 1024], ODT, tag="ob",
                                  name=f"ob{tb}")
                for eb in range(2):
                    cp_engines[eb](ob[:, 512 * eb:512 * eb + 512],
                                   pss[eb][:])
                    if split_dma:
                        nc.sync.dma_start(
                            out_d[128 * tb:128 * tb + 128,
                                  512 * eb:512 * eb + 512],
                            ob[:, 512 * eb:512 * eb + 512])
                if not split_dma:
                    nc.sync.dma_start(out_d[128 * tb:128 * tb + 128, :],
                                      ob[:])

            cpv = (nc.vector.tensor_copy, nc.vector.tensor_copy)

            def fq(tch, eb):
                return lambda: emit_qk_group(tch, eb, wq_s, qT_t)

            def fk(tch, eb):
                return lambda: emit_qk_group(tch, eb, wk_s, kT_t)

            def fv(tb):
                return lambda: emit_v_group(tb)

            def f5(tb):
                return lambda: emit_stage5(ps1_pool, tb, cpv)

            # stage-1 chunk 0 up front (DMA-paced); everything else is
            # threaded through the pairs' j-loops as fillers. Dummy
            # matmul bursts on the tiny, first-to-land ones64 tile run
            # ahead of and between the first groups: they bridge the
            # DMA-wait gaps so the HAM clock gate warms to K=8/8 early
            # and stays there (idle gaps re-throttle to half clock).
            wu = pst_pool.tile([128, 2 * CH], F32, tag="pst", name="warm")
            state["wui"] = 0

            def warm_spin(n):
                i0 = state["wui"]
                for i in range(i0, i0 + n):
                    nc.tensor.matmul(wu[0:64, 0:64], ones64_t[:, 0:64],
                                     ones64_t[:, 0:64], start=(i == 0),
                                     stop=False, skip_group_check=True)
                state["wui"] = i0 + n

            warm_spin(30)
            emit_stage1 = [("q", 0, 0), ("k", 0, 0), ("q", 0, 1),
                           ("k", 0, 1)]
            for kind, tch, eb in emit_stage1:
                emit_qk_group(tch, eb, wq_s if kind == "q" else wk_s,
                              qT_t if kind == "q" else kT_t)
                warm_spin(12)
            for tb in range(4):
                emit_v_group(tb)
                warm_spin(12)
            nc.tensor.matmul(wu[0:64, 0:64], ones64_t[:, 0:64],
                             ones64_t[:, 0:64], start=False, stop=True,
                             skip_group_check=True)

            # filler allocation balances each pair's serial exp chain
            # (ACT, ~1.3us per 2-j block) against its own PE work plus
            # fillers, so the PE never stalls on the pst-buffer/exp
            # dependency. Late pairs have the longest exp chains and the
            # least inherent PE work, so they get the most fillers.
            emit_pair(0, 0, [fq(1, 0), fq(1, 1), fk(1, 0), fk(1, 1)])
            emit_pair(0, 1, [fv(4), fv(5), fv(6), fv(7)])
            emit_pair(1, 0, [fq(2, 0), fq(2, 1), fk(2, 0), fk(2, 1)])
            emit_pair(1, 1, [fv(8), fv(9), fv(10), fv(11)])
            emit_pair(2, 0, [fq(3, 0), fq(3, 1), fk(3, 0), fk(3, 1),
                             fv(12), fv(13), fv(14), fv(15)])
            emit_pair(2, 1, [f5(0), f5(1), f5(2), f5(3)])
            emit_pair(3, 0, [f5(4), f5(5), f5(6), f5(7)])
            emit_pair(3, 1, [f5(8), f5(9), f5(10), f5(11)])

        # tail: exp stream is done, so ACT picks up half the copies, and
        # a deeper psum pool (attention pools closed above) keeps two
        # t-blocks in flight.
        with tc.tile_pool(name="ps5b", bufs=4, space="PSUM") as ps5b_pool:
            cps = (nc.vector.tensor_copy, nc.scalar.copy)
            for tb in range(12, 16):
                emit_stage5(ps5b_pool, tb, cps, split_dma=True)

    nc.compile()
    return nc


_PROG = None


def _get_prog():
    global _PROG
    if _PROG is None:
        _PROG = build_program()
    return _PROG


def make_in_maps(x, Q, K, V, W_o):
    np_dt = mybir.dt.np(WDT)
    B = x.shape[0]
    maskz = np.greater_equal(np.arange(128)[None, :],
                             np.arange(128)[:, None]).astype(np.float32)
    maskz = maskz.astype(np_dt)

    def pack_rows(w, cols):  # [1024, cols] -> [128, 8*cols]
        return np.ascontiguousarray(
            w.reshape(8, 128, cols).transpose(1, 0, 2).reshape(128, 8 * cols))

    in_maps = []
    for c in range(N_CORES):
        b, g = divmod(c, N_CORES // B)
        sl = slice(DS * g, DS * g + DS)
        xT = np.ascontiguousarray(x[b].T)  # [1024, 2048]
        # xp[p, 4096c + 512db + t] = xT[128db+p, 512c+t]
        xp = (xT.reshape(8, 128, 4, 512).transpose(1, 2, 0, 3)
              .reshape(128, 16384))
        wop = W_o[:, sl].T  # [256, 1024]
        wop = (wop.reshape(2, 128, 1024).transpose(1, 0, 2)
               .reshape(128, 2048))
        in_maps.append({
            "xp": np.ascontiguousarray(xp).astype(np_dt),
            "wqp": pack_rows(Q[sl, :].T, DS).astype(np_dt),
            "wkp": pack_rows(K[sl, :].T, DS).astype(np_dt),
            "wvp": pack_rows(V[sl, :].T, DS).astype(np_dt),
            "wop": np.ascontiguousarray(wop).astype(np_dt),
            "maskz": maskz,
        })
    return in_maps


def kernel(x, Q, K, V, W_o):
    x = np.asarray(x, dtype=np.float32)
    Q = np.asarray(Q, dtype=np.float32)
    K = np.asarray(K, dtype=np.float32)
    V = np.asarray(V, dtype=np.float32)
    W_o = np.asarray(W_o, dtype=np.float32)

    nc = _get_prog()
    in_maps = make_in_maps(x, Q, K, V, W_o)
    res = run_bass_kernel_spmd(nc, in_maps, core_ids=list(range(N_CORES)))

    B = x.shape[0]
    out = np.zeros((B, T, D), dtype=np.float32)
    for c in range(N_CORES):
        out[c // (N_CORES // B)] += np.asarray(res.results[c]["out"],
                                               dtype=np.float32)
    return out
